# revision 1
# baseline (speedup 1.0000x reference)
"""Trainium2 Bass kernel for nn_ConvGraph_SC (gnn_message_passing).

Reference computation (per batch b of 64, N=32 nodes, C=512 channels, 7x7 spatial):
    state = input.mean(axis=(3,4))                       # [B, N, C]
    mat1  = state @ W1.T + b1
    mat2  = state @ W2.T + b2
    adj   = mat1 @ mat2.T                                # [B, N, N]
    soft  = softmax((adj - mean(adj)) / std(adj), rows)  # global mean/std, ddof=1
    out   = mean(soft @ state + state, axis=1)           # [B, C]

Device-side algebra:
  * adj = S A S^T + su 1^T + 1 sv^T + c0, with A = W1^T W2, u = W1^T b2,
    v = W2^T b1, c0 = b1.b2 precomputed on host -> one [C,C] GEMM instead of two.
  * Row softmax is invariant to row-constant shifts -> su, c0 and the global
    mean drop out of the softmax; they only enter the mean/std statistics,
    which are computed from per-row sums with closed-form corrections.
  * 1/std via Newton rsqrt on the vector engine (magic-seed + 3 iterations)
    so the scalar engine only ever needs the exp table set (one table load).
  * out[b,c] = (1/N) sum_m (colsum(soft)[m] + 1) * state[m,c] -> a single
    weighted column sum; no new_state materialization.
  * Spatial mean scale 1/49 folded into host-scaled A, u, v; 1/(N*49) into
    the final weight vector.

Pipeline: everything except the grouped stats/softmax chain is emitted per
batch so it overlaps the DMA stream; TA = S A uses st^T as the (32-column)
stationary operand against the full 512-wide A slab to keep fp32 LDWEIGHTS
off the critical path.

Sharding: pure data parallel, 8 batches per NeuronCore, weights replicated.
"""

import numpy as np

import concourse.bacc as bacc
import concourse.tile as tile
from concourse import masks, mybir
from concourse.bass_utils import run_bass_kernel_spmd

F32 = mybir.dt.float32
U32 = mybir.dt.uint32
I32 = mybir.dt.int32
NCORES = 8
B, N, C, HW = 64, 32, 512, 49
BPC = B // NCORES          # batches per core
FREE = N * C * HW // 128   # 6272 floats per partition per batch
HALF = FREE // 2           # 3136
G = 4                      # batches per stats group
NG = BPC // G              # groups per core
K1023 = float(np.sqrt(np.float64(1023.0)))

_CACHED_NC = None

A_ = mybir.AluOpType


def build_bass(debug=False):
    nc = bacc.Bacc("TRN2", target_bir_lowering=False)

    x_d = nc.declare_dram_parameter("x", [BPC, 128, FREE], F32, isOutput=False)
    a_d = nc.declare_dram_parameter("amat", [C, C], F32, isOutput=False)
    uv_d = nc.declare_dram_parameter("uv", [C, 2], F32, isOutput=False)
    c0_d = nc.declare_dram_parameter("c0", [32, 1], F32, isOutput=False)
    out_d = nc.declare_dram_parameter("out", [128, 4 * BPC], F32, isOutput=True)
    if debug:
        dbg_st = nc.declare_dram_parameter("dbg_st", [128, 128 * BPC], F32, True)
        dbg_ta = nc.declare_dram_parameter("dbg_ta", [128, 1024], F32, True)
        dbg_adj = nc.declare_dram_parameter("dbg_adj", [32, 128 * NG], F32, True)
        dbg_suv = nc.declare_dram_parameter("dbg_suv", [2, 128 * NG], F32, True)
        dbg_sg = nc.declare_dram_parameter("dbg_sg", [32, 16 * NG], F32, True)
        dbg_inv = nc.declare_dram_parameter("dbg_inv", [32, G * NG], F32, True)
        dbg_wf = nc.declare_dram_parameter("dbg_wf", [1, 128 * NG], F32, True)

    with tile.TileContext(nc) as tc:
        with (
            tc.tile_pool(name="xpool", bufs=6) as xpool,
            tc.tile_pool(name="singles", bufs=1) as singles,
            tc.tile_pool(name="srawp", bufs=3) as srawp,
            tc.tile_pool(name="tanat", bufs=2) as tanat_pool,
            tc.tile_pool(name="small", bufs=2) as small,
            tc.tile_pool(name="ps_t", bufs=1, space="PSUM") as ps_t_pool,
            tc.tile_pool(name="ps_ta", bufs=1, space="PSUM") as ps_ta_pool,
            tc.tile_pool(name="ps_tt", bufs=1, space="PSUM") as ps_tt_pool,
            tc.tile_pool(name="ps_adj", bufs=2, space="PSUM") as ps_adj_pool,
            tc.tile_pool(name="ps_sm", bufs=2, space="PSUM") as ps_sm_pool,
            tc.tile_pool(name="ps_wb", bufs=1, space="PSUM") as ps_wb_pool,
        ):
            # ---- persistent tiles -----------------------------------------
            ident = singles.tile([128, 128], F32)
            ones_col = singles.tile([32, 1], F32)
            ones_row = singles.tile([1, 128], F32)
            a_sb = singles.tile([128, 4 * C], F32)
            uv_sb = singles.tile([128, 8], F32)
            c0_sb = singles.tile([32, 1], F32)
            # state^T for all batches: col 128b + 4k + r <-> (n=k, c=128r+j)
            st_all = singles.tile([128, 128 * BPC], F32)
            # (S A)^T for all batches: col 256s + 32b + k <-> (d=128s+j, n=k)
            ta_all = singles.tile([128, 1024], F32)
            outsb = singles.tile([128, 4 * BPC], F32)

            def load_weights():
                # emitted after the first batch's x DMAs so the input stream
                # owns the head of the DMA queues
                for r in range(4):
                    nc.sync.dma_start(
                        out=a_sb[:, 512 * r : 512 * (r + 1)],
                        in_=a_d[128 * r : 128 * (r + 1), :],
                    )
                for r in range(4):
                    nc.sync.dma_start(
                        out=uv_sb[:, 2 * r : 2 * (r + 1)],
                        in_=uv_d[128 * r : 128 * (r + 1), :],
                    )
                nc.sync.dma_start(out=c0_sb[:], in_=c0_d[:])
                masks.make_identity(nc, ident[:])
                nc.vector.memset(ones_col[:], 1.0)
                nc.vector.memset(ones_row[:], 1.0)

            for g in range(NG):
                ps_sm = ps_sm_pool.tile([128, 512], F32)
                # ps_sm regions (one bank): su rows [:1, 0:128],
                # sv rows [:1, 128:256], su_col [:32, 256:260],
                # stats1 [:1, 260:268], statsbc [:32, 268:276]
                ps_adj = ps_adj_pool.tile([32, 128], F32)
                su_sb = small.tile([1, 128], F32, tag="su_sb")
                sv_sb = small.tile([1, 128], F32, tag="sv_sb")

                for bp in range(G):
                    b = G * g + bp
                    # -- load + spatial sum + transpose ---------------------
                    sraw = srawp.tile([128, 128], F32, tag="sraw")
                    for h in range(2):
                        xb = xpool.tile([128, HALF], F32, tag="xb")
                        nc.sync.dma_start(
                            out=xb[:], in_=x_d[b, :, HALF * h : HALF * (h + 1)]
                        )
                        nc.vector.reduce_sum(
                            out=sraw[:, 64 * h : 64 * (h + 1)],
                            in_=xb[:].rearrange("p (q s) -> p q s", s=HW),
                            axis=mybir.AxisListType.X,
                        )
                    if b == 0:
                        load_weights()
                    ps_t = ps_t_pool.tile([128, 128], F32)
                    nc.tensor.transpose(ps_t[:], sraw[:], ident[:])
                    nc.scalar.copy(st_all[:, 128 * b : 128 * (b + 1)], ps_t[:])

                    # -- TA = S A (natural layout), st^T as stationary ------
                    ps_ta = ps_ta_pool.tile([32, 512], F32)
                    for r in range(4):
                        nc.tensor.matmul(
                            ps_ta[:],
                            st_all[:, 128 * b + r : 128 * (b + 1) : 4],
                            a_sb[:, 512 * r : 512 * (r + 1)],
                            start=(r == 0), stop=(r == 3),
                        )
                    ta_nat = tanat_pool.tile([32, 512], F32, tag="ta_nat")
                    nc.scalar.copy(ta_nat[:], ps_ta[:])

                    # transpose TA -> TA^T blocks [128, 32] per d-block s
                    ps_tt = ps_tt_pool.tile([128, 128], F32)
                    for s in range(4):
                        nc.tensor.transpose(
                            ps_tt[:, 32 * s : 32 * (s + 1)],
                            ta_nat[:, 128 * s : 128 * (s + 1)],
                            ident[:32, :32],
                        )
                    nc.scalar.copy(
                        ta_all[:].rearrange("p (s q) -> p s q", q=256)[
                            :, :, 32 * b : 32 * (b + 1)
                        ],
                        ps_tt[:].rearrange("p (s k) -> p s k", k=32),
                    )

                    # -- su/sv rows for this batch --------------------------
                    for r in range(4):
                        nc.tensor.matmul(
                            ps_sm[:1, 32 * bp : 32 * (bp + 1)],
                            uv_sb[:, 2 * r : 2 * r + 1],
                            st_all[:, 128 * b + r : 128 * (b + 1) : 4],
                            start=(r == 0), stop=(r == 3),
                        )
                    for r in range(4):
                        nc.tensor.matmul(
                            ps_sm[:1, 128 + 32 * bp : 128 + 32 * (bp + 1)],
                            uv_sb[:, 2 * r + 1 : 2 * r + 2],
                            st_all[:, 128 * b + r : 128 * (b + 1) : 4],
                            start=(r == 0), stop=(r == 3),
                        )
                    nc.vector.tensor_copy(
                        su_sb[:, 32 * bp : 32 * (bp + 1)],
                        ps_sm[:1, 32 * bp : 32 * (bp + 1)],
                    )
                    nc.vector.tensor_copy(
                        sv_sb[:, 32 * bp : 32 * (bp + 1)],
                        ps_sm[:1, 128 + 32 * bp : 128 + 32 * (bp + 1)],
                    )
                    # su as a column: [32, 1] at ps_sm[:32, 256+bp]
                    nc.tensor.matmul(
                        ps_sm[:32, 256 + bp : 257 + bp],
                        su_sb[0:1, 32 * bp : 32 * (bp + 1)],
                        ones_row[:1, 0:1],
                        start=True, stop=True,
                    )

                    # -- adjacency (minus row-constants): a + 1 sv^T --------
                    asl = slice(32 * bp, 32 * (bp + 1))
                    for s in range(4):
                        nc.tensor.matmul(
                            ps_adj[:, asl],
                            ta_all[:, 256 * s + 32 * b : 256 * s + 32 * (b + 1)],
                            st_all[:, 128 * b + s : 128 * (b + 1) : 4],
                            start=(s == 0), stop=False,
                        )
                    nc.tensor.matmul(
                        ps_adj[:, asl],
                        ones_row[:1, 0:32],
                        sv_sb[0:1, 32 * bp : 32 * (bp + 1)],
                        start=False, stop=True,
                    )

                # ---- grouped stats: S1/S2 of TRUE adj via row sums --------
                q_g = small.tile([32, G], F32, tag="q_g")
                nc.vector.tensor_scalar(
                    out=q_g[:], in0=ps_sm[:32, 256 : 256 + G],
                    scalar1=c0_sb[:], scalar2=None, op0=A_.add,
                )
                t_g = small.tile([32, G], F32, tag="t_g")
                nc.vector.reduce_sum(
                    out=t_g[:],
                    in_=ps_adj[:].rearrange("p (b m) -> p b m", m=32),
                    axis=mybir.AxisListType.X,
                )
                rowsq = small.tile([32, G], F32, tag="rowsq")
                sq_scr = small.tile([32, 32], F32, tag="sq_scr")
                for bp in range(G):
                    nc.scalar.activation(
                        out=sq_scr[:], in_=ps_adj[:, 32 * bp : 32 * (bp + 1)],
                        func=mybir.ActivationFunctionType.Square,
                        accum_out=rowsq[:, bp : bp + 1],
                    )
                # stats_g: cols 0:G = S1 rows, G:2G = S2 rows (true adj)
                stats_g = small.tile([32, 2 * G], F32, tag="stats_g")
                q32 = small.tile([32, G], F32, tag="q32")
                nc.vector.tensor_scalar(
                    out=q32[:], in0=q_g[:], scalar1=32.0, scalar2=None,
                    op0=A_.mult,
                )
                nc.vector.tensor_add(stats_g[:, 0:G], q32[:], t_g[:])
                # S2row = rowsq + q*(2t + 32q); 2t + 32q = t + S1row
                h_g = small.tile([32, G], F32, tag="h_g")
                nc.vector.tensor_add(h_g[:], t_g[:], stats_g[:, 0:G])
                s2c = small.tile([32, G], F32, tag="s2c")
                nc.vector.tensor_mul(s2c[:], q_g[:], h_g[:])
                nc.vector.tensor_add(stats_g[:, G : 2 * G], rowsq[:], s2c[:])

                # cross-partition sum + broadcast back (PE ones trick)
                nc.tensor.matmul(
                    ps_sm[:1, 260:268], ones_col[:], stats_g[:],
                    start=True, stop=True,
                )
                s_sb = small.tile([1, 2 * G], F32, tag="s_sb")
                nc.vector.tensor_copy(s_sb[:], ps_sm[:1, 260:268])
                nc.tensor.matmul(
                    ps_sm[:32, 268:276], ones_row[:1, 0:32], s_sb[:],
                    start=True, stop=True,
                )
                s_all = small.tile([32, 2 * G], F32, tag="s_all")
                nc.vector.tensor_copy(s_all[:], ps_sm[:32, 268:276])

                # ---- inv_std = sqrt(1023)/sqrt(S2 - S1^2/1024) ------------
                t1 = small.tile([32, G], F32, tag="t1")
                nc.vector.tensor_mul(t1[:], s_all[:, 0:G], s_all[:, 0:G])
                nc.vector.tensor_scalar(
                    out=t1[:], in0=t1[:], scalar1=-1.0 / 1024.0, scalar2=None,
                    op0=A_.mult,
                )
                v1023 = small.tile([32, G], F32, tag="v1023")
                nc.vector.tensor_add(v1023[:], t1[:], s_all[:, G : 2 * G])
                # Newton rsqrt with magic seed: bits = C - (iv >> 1).
                # DVE int ALU is float-rounded; seed only needs ~3% accuracy.
                yint = small.tile([32, G], I32, tag="yint")
                nc.vector.tensor_scalar(
                    out=yint[:], in0=v1023[:].bitcast(I32), scalar1=1,
                    scalar2=None, op0=A_.logical_shift_right,
                )
                nc.vector.tensor_scalar(
                    out=yint[:], in0=yint[:], scalar1=-1,
                    scalar2=0x5F3759DF, op0=A_.mult, op1=A_.add,
                )
                y = small.tile([32, G], F32, tag="y")
                nc.vector.tensor_copy(y[:], yint[:].bitcast(F32))
                ya = small.tile([32, G], F32, tag="ya")
                yb = small.tile([32, G], F32, tag="yb")
                for it in range(3):
                    nc.vector.tensor_mul(ya[:], y[:], y[:])
                    nc.vector.tensor_mul(yb[:], ya[:], v1023[:])
                    last = it == 2
                    nc.vector.tensor_scalar(
                        out=ya[:], in0=yb[:],
                        scalar1=(-0.5 * K1023) if last else -0.5,
                        scalar2=(1.5 * K1023) if last else 1.5,
                        op0=A_.mult, op1=A_.add,
                    )
                    nc.vector.tensor_mul(y[:], y[:], ya[:])
                inv_g = y  # [32, G] inv_std per batch column

                # ---- softmax pieces --------------------------------------
                negmax = small.tile([32, G], F32, tag="negmax")
                nc.vector.reduce_max(
                    out=negmax[:],
                    in_=ps_adj[:].rearrange("p (b m) -> p b m", m=32),
                    axis=mybir.AxisListType.X, negate=True,
                )
                negm = small.tile([32, G], F32, tag="negm")
                nc.vector.tensor_mul(negm[:], negmax[:], inv_g[:])
                expt = small.tile([32, 32 * G], F32, tag="expt")
                rowsum = small.tile([32, G], F32, tag="rowsum")
                for bp in range(G):
                    nc.scalar.activation(
                        out=expt[:, 32 * bp : 32 * (bp + 1)],
                        in_=ps_adj[:, 32 * bp : 32 * (bp + 1)],
                        func=mybir.ActivationFunctionType.Exp,
                        bias=negm[:, bp : bp + 1], scale=inv_g[:, bp : bp + 1],
                        accum_out=rowsum[:, bp : bp + 1],
                    )
                recip = small.tile([32, G], F32, tag="recip")
                nc.vector.reciprocal(recip[:], rowsum[:])

                # w[k] = colsum(soft) per batch: [1, 32] each at partition 0
                # ps_wb regions: w rows [:1, 0:128], wb bcast [:, 128:256]
                ps_wb = ps_wb_pool.tile([128, 256], F32)
                for bp in range(G):
                    nc.tensor.matmul(
                        ps_wb[:1, 32 * bp : 32 * (bp + 1)],
                        recip[:, bp : bp + 1],
                        expt[:, 32 * bp : 32 * (bp + 1)],
                        start=True, stop=True,
                    )
                wf = small.tile([1, 128], F32, tag="wf")
                nc.vector.tensor_scalar(
                    out=wf[:], in0=ps_wb[:1, 0:128],
                    scalar1=1.0 / (N * HW), scalar2=1.0 / (N * HW),
                    op0=A_.mult, op1=A_.add,
                )
                # broadcast each batch's weight row to 128 partitions
                for bp in range(G):
                    nc.tensor.matmul(
                        ps_wb[:, 128 + 32 * bp : 128 + 32 * (bp + 1)],
                        ones_row[:],
                        wf[0:1, 32 * bp : 32 * (bp + 1)],
                        start=True, stop=True,
                    )

                # ---- epilogue: out[c] = sum_k st^T[c, k] * wf[k] ----------
                gcols = slice(512 * g, 512 * (g + 1))
                for r in range(4):
                    scr = small.tile([128, 128], F32, tag="scr")
                    nc.vector.tensor_mul(
                        scr[:], st_all[:, gcols][:, r::4], ps_wb[:, 128:256]
                    )
                    nc.vector.reduce_sum(
                        out=outsb[:, 16 * g + r : 16 * g + 16 : 4],
                        in_=scr[:].rearrange("p (b m) -> p b m", m=32),
                        axis=mybir.AxisListType.X,
                    )
                nc.sync.dma_start(
                    out=out_d[:, 16 * g : 16 * (g + 1)],
                    in_=outsb[:, 16 * g : 16 * (g + 1)],
                )

                if debug:
                    adj_dbg = small.tile([32, 128], F32, tag="adj_dbg")
                    nc.vector.tensor_copy(adj_dbg[:], ps_adj[:])
                    nc.sync.dma_start(
                        out=dbg_adj[:, 128 * g : 128 * (g + 1)], in_=adj_dbg[:]
                    )
                    nc.sync.dma_start(
                        out=dbg_suv[0:1, 128 * g : 128 * (g + 1)], in_=su_sb[:]
                    )
                    nc.sync.dma_start(
                        out=dbg_suv[1:2, 128 * g : 128 * (g + 1)], in_=sv_sb[:]
                    )
                    nc.sync.dma_start(
                        out=dbg_sg[:, 16 * g : 16 * g + 8], in_=stats_g[:]
                    )
                    nc.sync.dma_start(
                        out=dbg_sg[:, 16 * g + 8 : 16 * (g + 1)], in_=s_all[:]
                    )
                    nc.sync.dma_start(
                        out=dbg_inv[:, G * g : G * (g + 1)], in_=inv_g[:]
                    )
                    nc.sync.dma_start(
                        out=dbg_wf[:, 128 * g : 128 * (g + 1)], in_=wf[:]
                    )

            if debug:
                nc.sync.dma_start(out=dbg_st[:], in_=st_all[:])
                nc.sync.dma_start(out=dbg_ta[:], in_=ta_all[:])

    nc.finalize()
    return nc


def host_prep(input, W1, b1, W2, b2):
    input = np.ascontiguousarray(input, dtype=np.float32)
    w1 = np.asarray(W1, dtype=np.float64)
    w2 = np.asarray(W2, dtype=np.float64)
    b1 = np.asarray(b1, dtype=np.float64)
    b2 = np.asarray(b2, dtype=np.float64)
    amat = np.ascontiguousarray((w1.T @ w2) / (HW * HW), dtype=np.float32)
    u = (w1.T @ b2) / HW
    v = (w2.T @ b1) / HW
    uv = np.ascontiguousarray(np.stack([u, v], axis=1), dtype=np.float32)
    c0 = np.full((32, 1), float(b1 @ b2), dtype=np.float32)
    return input, amat, uv, c0


def kernel(input, W1, b1, W2, b2):
    global _CACHED_NC
    if _CACHED_NC is None:
        _CACHED_NC = build_bass()
    nc = _CACHED_NC

    input, amat, uv, c0 = host_prep(input, W1, b1, W2, b2)

    in_maps = []
    for i in range(NCORES):
        shard = input[BPC * i : BPC * (i + 1)].reshape(BPC, 128, FREE)
        in_maps.append({"x": shard, "amat": amat, "uv": uv, "c0": c0})

    res = run_bass_kernel_spmd(nc, in_maps, list(range(NCORES)))

    out = np.empty((B, C), dtype=np.float32)
    for i in range(NCORES):
        o = res.results[i]["out"]  # [128, 4*BPC], col = 4b + r
        out[BPC * i : BPC * (i + 1)] = (
            o.reshape(128, BPC, 4).transpose(1, 2, 0).reshape(BPC, C)
        )
    return out



# revision 8
# speedup vs baseline: 1.6412x; 1.6412x over previous
"""Trainium2 Bass kernel for nn_ConvGraph_SC (gnn_message_passing).

Reference computation (per batch b of 64, N=32 nodes, C=512 channels, 7x7 spatial):
    state = input.mean(axis=(3,4))                       # [B, N, C]
    mat1  = state @ W1.T + b1
    mat2  = state @ W2.T + b2
    adj   = mat1 @ mat2.T                                # [B, N, N]
    soft  = softmax((adj - mean(adj)) / std(adj), rows)  # global mean/std, ddof=1
    out   = mean(soft @ state + state, axis=1)           # [B, C]

Device-side algebra (same as v1):
  * adj = S A S^T + su 1^T + 1 sv^T + c0, with A = W1^T W2, u = W1^T b2,
    v = W2^T b1, c0 = b1.b2 precomputed on host -> one [C,C] GEMM.
  * Row softmax is invariant to row-constant shifts -> su, c0 and the global
    mean drop out; they only enter the mean/std statistics, computed from
    per-row sums with closed-form corrections.
  * 1/std via Newton rsqrt on the vector engine (magic seed + 2 iterations).
  * out[b,c] = (1/N) sum_m (colsum(soft)[m] + 1) * state[m,c].

v2 performance restructure (the big wins):
  * Input cast to fp16 on host -> DMA bytes halved (12.8 MB/core, ~30 us at
    the 435 GB/s per-core cap) and 1 cycle/col PE matmuls.
  * Host permutes each batch to [128 part = (n, c_high), (half, s, c_low64)]
    so the 49-spatial sum is done ON THE PE: 7 accumulating identity-matmuls
    of 448 contiguous fp16 columns -> PSUM [128,(7,64)], then one DVE reduce
    of FD=448 (DVE tensor_reduce only has a 1x uop, so the old FD=3136
    reduce at 3.4 us/half was the critical path).
  * TA^T = A^T S_g^T computed per GROUP of 4 batches with 128x128 A-blocks
    as stationary (full M=128 PE rows, fp16) instead of per-batch M=32 fp32
    matmuls (~5 cyc/col in LOW_HIGH mode).

Sharding: pure data parallel, 8 batches per NeuronCore, weights replicated.
"""

import numpy as np

import concourse.bacc as bacc
import concourse.tile as tile
from concourse import masks, mybir
from concourse.bass_utils import run_bass_kernel_spmd

F32 = mybir.dt.float32
F16 = mybir.dt.float16
I32 = mybir.dt.int32
NCORES = 8
B, N, C, HW = 64, 32, 512, 49
BPC = B // NCORES          # batches per core
FREE = N * C * HW // 128   # 6272 cols per partition per batch
HALF = FREE // 2           # 3136 = 49 * 64
G = 4                      # batches per stats group
NG = BPC // G              # groups per core
K1023 = float(np.sqrt(np.float64(1023.0)))

_CACHED_NC = None

A_ = mybir.AluOpType


def build_bass(debug=False):
    nc = bacc.Bacc("TRN2", target_bir_lowering=False)

    # x layout per batch: partition p = 4n + c_high (c_high = c >> 7),
    # col = 3136*h + 64*s + cl  with  c = 128*c_high + 64*h + cl, s in [0,49)
    x_d = nc.declare_dram_parameter("x", [BPC, 128, FREE], F16, isOutput=False)
    a_d = nc.declare_dram_parameter("amat", [C, C], F16, isOutput=False)
    uv_d = nc.declare_dram_parameter("uv", [C, 2], F16, isOutput=False)
    c0_d = nc.declare_dram_parameter("c0", [32, 1], F32, isOutput=False)
    out_d = nc.declare_dram_parameter("out", [128, 4 * BPC], F32, isOutput=True)
    if debug:
        dbg_st2 = nc.declare_dram_parameter("dbg_st2", [128, 1024], F16, True)
        dbg_ta = nc.declare_dram_parameter("dbg_ta", [128, 512 * NG], F16, True)
        dbg_adj = nc.declare_dram_parameter("dbg_adj", [32, 128 * NG], F32, True)
        dbg_suv = nc.declare_dram_parameter("dbg_suv", [2, 128 * NG], F16, True)
        dbg_sg = nc.declare_dram_parameter("dbg_sg", [32, 16 * NG], F32, True)
        dbg_inv = nc.declare_dram_parameter("dbg_inv", [32, G * NG], F32, True)
        dbg_wf = nc.declare_dram_parameter("dbg_wf", [1, 128 * NG], F16, True)

    with tile.TileContext(nc) as tc:
        with (
            nc.allow_low_precision(
                reason="fp16 intermediates; 2e-2 output tolerance"
            ),
            tc.tile_pool(name="xpool", bufs=3) as xpool,
            tc.tile_pool(name="singles", bufs=1) as singles,
            tc.tile_pool(name="srawp", bufs=3) as srawp,
            tc.tile_pool(name="tasbp", bufs=2) as tasbp,
            tc.tile_pool(name="small", bufs=2) as small,
            tc.tile_pool(name="ps_x", bufs=2, space="PSUM") as ps_x_pool,
            tc.tile_pool(name="ps_t", bufs=1, space="PSUM") as ps_t_pool,
            tc.tile_pool(name="ps_tat", bufs=1, space="PSUM") as ps_tat_pool,
            tc.tile_pool(name="ps_adj", bufs=2, space="PSUM") as ps_adj_pool,
            tc.tile_pool(name="ps_sm", bufs=2, space="PSUM") as ps_sm_pool,
        ):
            # ---- persistent tiles -----------------------------------------
            ident16 = singles.tile([128, 128], F16)
            ones_r16 = singles.tile([1, 128], F16)
            ones_r32 = singles.tile([1, 128], F32)
            ones_c32 = singles.tile([32, 1], F32)
            # A blocks: a_sb[p, 512*r + d] = A[128*r + p, d]
            a_sb = singles.tile([128, 4 * 512], F16)
            uv_sb = singles.tile([128, 8], F16)
            c0_sb = singles.tile([32, 1], F32)
            # S^T slabs: st2[j, 256*r + 32*b + k] = Ssum_b[k, 128*r + j]
            st2 = singles.tile([128, 128 * BPC], F16)
            outsb = singles.tile([128, 4 * BPC], F32)

            def load_weights():
                # emitted after the first batch's x DMA so the input stream
                # owns the head of the DMA queues
                for r in range(4):
                    nc.sync.dma_start(
                        out=a_sb[:, 512 * r : 512 * (r + 1)],
                        in_=a_d[128 * r : 128 * (r + 1), :],
                    )
                for r in range(4):
                    nc.sync.dma_start(
                        out=uv_sb[:, 2 * r : 2 * (r + 1)],
                        in_=uv_d[128 * r : 128 * (r + 1), :],
                    )
                nc.sync.dma_start(out=c0_sb[:], in_=c0_d[:])
                masks.make_identity(nc, ident16[:])
                nc.vector.memset(ones_r16[:], 1.0)
                nc.vector.memset(ones_r32[:], 1.0)
                nc.vector.memset(ones_c32[:], 1.0)

            for g in range(NG):
                # ps_sm regions (one [128, 512] fp32 bank, per group):
                #   [:1, 0:128]     su rows; reused later for colsum(soft)
                #   [:1, 128:256]   sv rows
                #   [:32, 256:260]  su as columns, col = bp
                #   [:1, 260:268]   stats cross-partition sums (S1, S2)
                #   [:32, 268:276]  stats broadcast back
                #   [:, 276:404]    weight row broadcast to 128 partitions
                ps_sm = ps_sm_pool.tile([128, 512], F32)
                ps_adj = ps_adj_pool.tile([32, 128], F32)

                for bp in range(G):
                    b = G * g + bp
                    # -- load batch, spatial sum on PE, reduce, transpose ---
                    xb = xpool.tile([128, FREE], F16, tag="xb")
                    nc.sync.dma_start(out=xb[:], in_=x_d[b])
                    if b == 0:
                        load_weights()
                    sraw = srawp.tile([128, 128], F16, tag="sraw")
                    for h in range(2):
                        ps_xh = ps_x_pool.tile([128, 448], F32)
                        for a in range(7):
                            nc.tensor.matmul(
                                ps_xh[:],
                                ident16[:],
                                xb[:, HALF * h + 448 * a : HALF * h + 448 * (a + 1)],
                                start=(a == 0), stop=(a == 6),
                            )
                        # psum cols (s7, cl): sum the 7 s7 groups
                        nc.vector.reduce_sum(
                            out=sraw[:, 64 * h : 64 * (h + 1)],
                            in_=ps_xh[:].rearrange("p (s q) -> p q s", q=64),
                            axis=mybir.AxisListType.X,
                        )
                    ps_t = ps_t_pool.tile([128, 128], F16)
                    nc.tensor.transpose(ps_t[:], sraw[:], ident16[:])
                    # scatter: st2[j, (r, b, k)] <- ps_t[j, 4k + r]
                    nc.scalar.copy(
                        st2[:].rearrange("p (r x) -> p r x", x=256)[
                            :, :, 32 * b : 32 * (b + 1)
                        ],
                        ps_t[:].rearrange("p (k r) -> p r k", r=4),
                    )

                gcol = 128 * g  # st2 col offset of this group within a slab

                # -- TAT_s = sum_r A[r,s]^T @ S_g^T[r]  (all 4 batches) -----
                ps_tat = ps_tat_pool.tile([128, 512], F32)
                for s in range(4):
                    for r in range(4):
                        nc.tensor.matmul(
                            ps_tat[:, 128 * s : 128 * (s + 1)],
                            a_sb[:, 512 * r + 128 * s : 512 * r + 128 * (s + 1)],
                            st2[:, 256 * r + gcol : 256 * r + gcol + 128],
                            start=(r == 0), stop=(r == 3),
                        )
                ta_sb = tasbp.tile([128, 512], F16, tag="ta_sb")
                nc.scalar.copy(ta_sb[:], ps_tat[:])

                # -- su/sv rows for the whole group -------------------------
                for r in range(4):
                    nc.tensor.matmul(
                        ps_sm[:1, 0:128],
                        uv_sb[:, 2 * r : 2 * r + 1],
                        st2[:, 256 * r + gcol : 256 * r + gcol + 128],
                        start=(r == 0), stop=(r == 3),
                    )
                for r in range(4):
                    nc.tensor.matmul(
                        ps_sm[:1, 128:256],
                        uv_sb[:, 2 * r + 1 : 2 * r + 2],
                        st2[:, 256 * r + gcol : 256 * r + gcol + 128],
                        start=(r == 0), stop=(r == 3),
                    )
                su_sb = small.tile([1, 128], F16, tag="su_sb")
                sv_sb = small.tile([1, 128], F16, tag="sv_sb")
                nc.vector.tensor_copy(su_sb[:], ps_sm[:1, 0:128])
                nc.vector.tensor_copy(sv_sb[:], ps_sm[:1, 128:256])

                for bp in range(G):
                    b = G * g + bp
                    # su as a column: [32, 1] at ps_sm[:32, 256+bp]
                    nc.tensor.matmul(
                        ps_sm[:32, 256 + bp : 257 + bp],
                        su_sb[0:1, 32 * bp : 32 * (bp + 1)],
                        ones_r16[0:1, 0:1],
                        start=True, stop=True,
                    )
                    # adjacency (minus row-constants): S A S^T + 1 sv^T
                    asl = slice(32 * bp, 32 * (bp + 1))
                    for s in range(4):
                        nc.tensor.matmul(
                            ps_adj[:, asl],
                            ta_sb[:, 128 * s + 32 * bp : 128 * s + 32 * (bp + 1)],
                            st2[:, 256 * s + 32 * b : 256 * s + 32 * (b + 1)],
                            start=(s == 0), stop=False,
                        )
                    nc.tensor.matmul(
                        ps_adj[:, asl],
                        ones_r16[0:1, 0:32],
                        sv_sb[0:1, 32 * bp : 32 * (bp + 1)],
                        start=False, stop=True,
                    )

                # ---- grouped stats: S1/S2 of TRUE adj via row sums --------
                q_g = small.tile([32, G], F32, tag="q_g")
                nc.vector.tensor_scalar(
                    out=q_g[:], in0=ps_sm[:32, 256 : 256 + G],
                    scalar1=c0_sb[:], scalar2=None, op0=A_.add,
                )
                t_g = small.tile([32, G], F32, tag="t_g")
                nc.vector.reduce_sum(
                    out=t_g[:],
                    in_=ps_adj[:].rearrange("p (b m) -> p b m", m=32),
                    axis=mybir.AxisListType.X,
                )
                rowsq = small.tile([32, G], F32, tag="rowsq")
                sq_scr = small.tile([32, 32], F32, tag="sq_scr")
                for bp in range(G):
                    nc.scalar.activation(
                        out=sq_scr[:], in_=ps_adj[:, 32 * bp : 32 * (bp + 1)],
                        func=mybir.ActivationFunctionType.Square,
                        accum_out=rowsq[:, bp : bp + 1],
                    )
                # stats_g: cols 0:G = S1 rows, G:2G = S2 rows (true adj)
                stats_g = small.tile([32, 2 * G], F32, tag="stats_g")
                q32 = small.tile([32, G], F32, tag="q32")
                nc.vector.tensor_scalar(
                    out=q32[:], in0=q_g[:], scalar1=32.0, scalar2=None,
                    op0=A_.mult,
                )
                nc.vector.tensor_add(stats_g[:, 0:G], q32[:], t_g[:])
                # S2row = rowsq + q*(2t + 32q); 2t + 32q = t + S1row
                h_g = small.tile([32, G], F32, tag="h_g")
                nc.vector.tensor_add(h_g[:], t_g[:], stats_g[:, 0:G])
                s2c = small.tile([32, G], F32, tag="s2c")
                nc.vector.tensor_mul(s2c[:], q_g[:], h_g[:])
                nc.vector.tensor_add(stats_g[:, G : 2 * G], rowsq[:], s2c[:])

                # cross-partition sum + broadcast back (PE ones trick)
                nc.tensor.matmul(
                    ps_sm[:1, 260:268], ones_c32[:], stats_g[:],
                    start=True, stop=True,
                )
                s_sb = small.tile([1, 2 * G], F32, tag="s_sb")
                nc.vector.tensor_copy(s_sb[:], ps_sm[:1, 260:268])
                nc.tensor.matmul(
                    ps_sm[:32, 268:276], ones_r32[0:1, 0:32], s_sb[:],
                    start=True, stop=True,
                )
                s_all = small.tile([32, 2 * G], F32, tag="s_all")
                nc.vector.tensor_copy(s_all[:], ps_sm[:32, 268:276])

                # ---- inv_std = sqrt(1023)/sqrt(S2 - S1^2/1024) ------------
                t1 = small.tile([32, G], F32, tag="t1")
                nc.vector.tensor_mul(t1[:], s_all[:, 0:G], s_all[:, 0:G])
                nc.vector.tensor_scalar(
                    out=t1[:], in0=t1[:], scalar1=-1.0 / 1024.0, scalar2=None,
                    op0=A_.mult,
                )
                v1023 = small.tile([32, G], F32, tag="v1023")
                nc.vector.tensor_add(v1023[:], t1[:], s_all[:, G : 2 * G])
                # Newton rsqrt with magic seed: bits = C - (iv >> 1).
                yint = small.tile([32, G], I32, tag="yint")
                nc.vector.tensor_scalar(
                    out=yint[:], in0=v1023[:].bitcast(I32), scalar1=1,
                    scalar2=None, op0=A_.logical_shift_right,
                )
                nc.vector.tensor_scalar(
                    out=yint[:], in0=yint[:], scalar1=-1,
                    scalar2=0x5F3759DF, op0=A_.mult, op1=A_.add,
                )
                y = small.tile([32, G], F32, tag="y")
                nc.vector.tensor_copy(y[:], yint[:].bitcast(F32))
                ya = small.tile([32, G], F32, tag="ya")
                yb = small.tile([32, G], F32, tag="yb")
                for it in range(2):
                    nc.vector.tensor_mul(ya[:], y[:], y[:])
                    nc.vector.tensor_mul(yb[:], ya[:], v1023[:])
                    last = it == 1
                    nc.vector.tensor_scalar(
                        out=ya[:], in0=yb[:],
                        scalar1=(-0.5 * K1023) if last else -0.5,
                        scalar2=(1.5 * K1023) if last else 1.5,
                        op0=A_.mult, op1=A_.add,
                    )
                    nc.vector.tensor_mul(y[:], y[:], ya[:])
                inv_g = y  # [32, G] inv_std per batch column

                # ---- softmax pieces --------------------------------------
                negmax = small.tile([32, G], F32, tag="negmax")
                nc.vector.reduce_max(
                    out=negmax[:],
                    in_=ps_adj[:].rearrange("p (b m) -> p b m", m=32),
                    axis=mybir.AxisListType.X, negate=True,
                )
                negm = small.tile([32, G], F32, tag="negm")
                nc.vector.tensor_mul(negm[:], negmax[:], inv_g[:])
                expt = small.tile([32, 32 * G], F32, tag="expt")
                rowsum = small.tile([32, G], F32, tag="rowsum")
                for bp in range(G):
                    nc.scalar.activation(
                        out=expt[:, 32 * bp : 32 * (bp + 1)],
                        in_=ps_adj[:, 32 * bp : 32 * (bp + 1)],
                        func=mybir.ActivationFunctionType.Exp,
                        bias=negm[:, bp : bp + 1], scale=inv_g[:, bp : bp + 1],
                        accum_out=rowsum[:, bp : bp + 1],
                    )
                recip = small.tile([32, G], F32, tag="recip")
                nc.vector.reciprocal(recip[:], rowsum[:])

                # w[m] = colsum(soft) per batch: rows at ps_sm[:1, 152:280]
                for bp in range(G):
                    nc.tensor.matmul(
                        ps_sm[:1, 32 * bp : 32 * (bp + 1)],
                        recip[:, bp : bp + 1],
                        expt[:, 32 * bp : 32 * (bp + 1)],
                        start=True, stop=True,
                    )
                wf = small.tile([1, 128], F16, tag="wf")
                nc.vector.tensor_scalar(
                    out=wf[:], in0=ps_sm[:1, 0:128],
                    scalar1=1.0 / (N * HW), scalar2=1.0 / (N * HW),
                    op0=A_.mult, op1=A_.add,
                )
                # broadcast the group's weight row to 128 partitions (rank-1)
                nc.tensor.matmul(
                    ps_sm[:, 276:404], ones_r16[0:1, :], wf[0:1, :],
                    start=True, stop=True,
                )
                wb_sb = small.tile([128, 128], F16, tag="wb_sb")
                nc.vector.tensor_copy(wb_sb[:], ps_sm[:, 276:404])

                # ---- epilogue: out[c] = sum_k st2[c, k] * w[k] ------------
                for r in range(4):
                    scr = small.tile([128, 128], F16, tag="scr")
                    nc.vector.tensor_mul(
                        scr[:], st2[:, 256 * r + gcol : 256 * r + gcol + 128],
                        wb_sb[:],
                    )
                    nc.vector.reduce_sum(
                        out=outsb[:, 16 * g + r : 16 * g + 16 : 4],
                        in_=scr[:].rearrange("p (b m) -> p b m", m=32),
                        axis=mybir.AxisListType.X,
                    )
                nc.sync.dma_start(
                    out=out_d[:, 16 * g : 16 * (g + 1)],
                    in_=outsb[:, 16 * g : 16 * (g + 1)],
                )

                if debug:
                    adj_dbg = small.tile([32, 128], F32, tag="adj_dbg")
                    nc.vector.tensor_copy(adj_dbg[:], ps_adj[:])
                    nc.sync.dma_start(
                        out=dbg_adj[:, 128 * g : 128 * (g + 1)], in_=adj_dbg[:]
                    )
                    nc.sync.dma_start(
                        out=dbg_ta[:, 512 * g : 512 * (g + 1)], in_=ta_sb[:]
                    )
                    nc.sync.dma_start(
                        out=dbg_suv[0:1, 128 * g : 128 * (g + 1)], in_=su_sb[:]
                    )
                    nc.sync.dma_start(
                        out=dbg_suv[1:2, 128 * g : 128 * (g + 1)], in_=sv_sb[:]
                    )
                    nc.sync.dma_start(
                        out=dbg_sg[:, 16 * g : 16 * g + 8], in_=stats_g[:]
                    )
                    nc.sync.dma_start(
                        out=dbg_sg[:, 16 * g + 8 : 16 * (g + 1)], in_=s_all[:]
                    )
                    nc.sync.dma_start(
                        out=dbg_inv[:, G * g : G * (g + 1)], in_=inv_g[:]
                    )
                    nc.sync.dma_start(
                        out=dbg_wf[0:1, 128 * g : 128 * (g + 1)], in_=wf[:]
                    )

            if debug:
                nc.sync.dma_start(out=dbg_st2[:], in_=st2[:])

    nc.finalize()
    return nc


def host_prep(input, W1, b1, W2, b2):
    # x: [B, N, C, 7, 7] -> per batch [128, (h, s, cl)] fp16
    #    partition p = 4n + c_high, col = 3136*h + 64*s + cl
    x16 = np.asarray(input, dtype=np.float16)
    xr = (
        x16.reshape(B, 32, 4, 2, 64, 49)
        .transpose(0, 1, 2, 3, 5, 4)
        .reshape(B, 128, FREE)
    )
    xr = np.ascontiguousarray(xr)
    w1 = np.asarray(W1, dtype=np.float64)
    w2 = np.asarray(W2, dtype=np.float64)
    b1 = np.asarray(b1, dtype=np.float64)
    b2 = np.asarray(b2, dtype=np.float64)
    amat = np.ascontiguousarray((w1.T @ w2) / (HW * HW), dtype=np.float16)
    u = (w1.T @ b2) / HW
    v = (w2.T @ b1) / HW
    uv = np.ascontiguousarray(np.stack([u, v], axis=1), dtype=np.float16)
    c0 = np.full((32, 1), float(b1 @ b2), dtype=np.float32)
    return xr, amat, uv, c0


def make_in_maps(input, W1, b1, W2, b2):
    xr, amat, uv, c0 = host_prep(input, W1, b1, W2, b2)
    in_maps = []
    for i in range(NCORES):
        shard = xr[BPC * i : BPC * (i + 1)]
        in_maps.append({"x": shard, "amat": amat, "uv": uv, "c0": c0})
    return in_maps


def kernel(input, W1, b1, W2, b2):
    global _CACHED_NC
    if _CACHED_NC is None:
        _CACHED_NC = build_bass()
    nc = _CACHED_NC

    in_maps = make_in_maps(input, W1, b1, W2, b2)
    res = run_bass_kernel_spmd(nc, in_maps, list(range(NCORES)))

    out = np.empty((B, C), dtype=np.float32)
    for i in range(NCORES):
        o = res.results[i]["out"]  # [128, 4*BPC], col = 4b + r
        out[BPC * i : BPC * (i + 1)] = (
            o.reshape(128, BPC, 4).transpose(1, 2, 0).reshape(BPC, C)
        )
    return out
